# revision 2
# baseline (speedup 1.0000x reference)
"""Trainium2 Bass kernel for CombinedKSpaceRowwiseMSELoss (v4).

loss = mean((pred-target)^2 over central cols) + mean(|pred-target| over
periphery cols), means over both channels jointly.

Pure data parallel over batch: 4 batches (5120 rows of 640 f32) per core,
sharded B across the 8 cores; host sums the per-core partials.

Perf notes (single-pass latency = HBM stream time + ramp + tail):
- The 26.2 MB/core of reads is HBM-bound (~358 GB/s/NC -> ~73 us floor), so
  the kernel optimizes the ramp/tail around an always-streaming DMA pipe.
- DMA tiling uses a DESCENDING rows-per-partition schedule (10,10,10,5,4,1):
  big early DMAs amortize descriptor posting; the tiny final tile keeps the
  serial end-of-stream chain (last DMA -> sub -> reduce -> result DMA) short.
- pred DMAs ride the SP HWDGE ring, target DMAs the ACT HWDGE ring. The ACT
  engine also runs the central Square+accum, and both instruction queues are
  in-order — so each tile's Square is emitted ONE TILE LATE (after the next
  tile's target dma_start). Descriptor posting therefore never waits on the
  current tile's compute and the SDMA engines always have queued work.
- io pool is triple-buffered (DMA posting runs ~2 tiles ahead); diff/sq
  scratch are single-buffered (their reuse hazards are same-engine serial,
  so extra buffers buy nothing).
- Tail: after the last (1-row) tile lands, the chain is just a 640-elem sub,
  two 240-elem abs-reduces (DVE) in parallel with a 160-elem Square (ACT),
  then the two tiny accumulator DMAs go out on SEPARATE rings so their
  completion receipts overlap.
"""

import sys

for _p in ("/opt/trn_rl_repo",):
    if _p not in sys.path:
        sys.path.insert(0, _p)

import numpy as np
from contextlib import ExitStack

import concourse.bass as bass
import concourse.tile as tile
from concourse import bacc, mybir
from concourse.bass_utils import run_bass_kernel_spmd

N_CORES = 8
B, C, H, W = 32, 2, 640, 640
B_SHARD = B // N_CORES          # 4 batch elements per core
ROWS = B_SHARD * C * H          # 5120 rows per core
P = 128                         # SBUF partitions
SCHED = (10, 10, 10, 5, 4, 1)   # rows/partition per tile (sum = ROWS/P = 40)
CW = int(W * 0.25)              # 160 central cols
CS = (W - CW) // 2              # 240
CE = CS + CW                    # 400
PW = W - CW                     # 480 periphery cols per row

F32 = mybir.dt.float32


def build_program(
    loop_n: int = 1,
    sched: tuple = SCHED,
    io_bufs: int = 3,
    work_bufs: int = 1,
    lag: int = 1,
) -> bass.Bass:
    assert sum(sched) * P == ROWS
    T = len(sched)
    rmax = max(sched)
    nc = bacc.Bacc("TRN2", target_bir_lowering=False, debug=False)

    pred = nc.dram_tensor("pred", [ROWS, W], F32, kind="ExternalInput")
    tgt = nc.dram_tensor("target", [ROWS, W], F32, kind="ExternalInput")
    cacc_out = nc.dram_tensor("cacc", [P, T], F32, kind="ExternalOutput")
    pacc_out = nc.dram_tensor("pacc", [P, 2 * T], F32, kind="ExternalOutput")

    with tile.TileContext(nc) as tc:
        with ExitStack() as ctx:
            io_pool = ctx.enter_context(tc.tile_pool(name="io", bufs=io_bufs))
            work_pool = ctx.enter_context(tc.tile_pool(name="work", bufs=work_bufs))
            acc_pool = ctx.enter_context(tc.tile_pool(name="acc", bufs=1))

            cacc = acc_pool.tile([P, T], F32)
            pacc = acc_pool.tile([P, 2 * T], F32)

            def emit_central(i, r, diffs):
                # ACT: cacc[:, i] = sum over (r, CW) of diff^2
                d3 = diffs[i][:, : r * W].rearrange("p (r w) -> p r w", w=W)
                sq = work_pool.tile([P, rmax * CW], F32, tag="sq")
                nc.scalar.activation(
                    sq[:, : r * CW].rearrange("p (r w) -> p r w", w=CW),
                    d3[:, :, CS:CE],
                    mybir.ActivationFunctionType.Square,
                    accum_out=cacc[:, i : i + 1],
                )

            def emit_tile(i, r, base, diffs):
                # DRAM rows [base, base + P*r): partition p holds rows
                # base + p*r .. base + (p+1)*r - 1 => one contiguous
                # r*W*4-byte descriptor per partition.
                src_p = pred.ap()[base : base + P * r].rearrange(
                    "(p r) w -> p (r w)", p=P
                )
                src_t = tgt.ap()[base : base + P * r].rearrange(
                    "(p r) w -> p (r w)", p=P
                )
                pt = io_pool.tile([P, rmax * W], F32, tag="pred")
                gt = io_pool.tile([P, rmax * W], F32, tag="tgt")
                nc.sync.dma_start(pt[:, : r * W], src_p)
                nc.scalar.dma_start(gt[:, : r * W], src_t)

                # Lagged Square for an earlier tile goes AFTER this tile's
                # target dma_start in the ACT queue.
                if i >= lag:
                    emit_central(i - lag, sched[i - lag], diffs)

                diff = work_pool.tile([P, rmax * W], F32, tag="diff")
                diffs[i] = diff
                nc.vector.tensor_sub(diff[:, : r * W], pt[:, : r * W], gt[:, : r * W])
                d3 = diff[:, : r * W].rearrange("p (r w) -> p r w", w=W)

                # DVE: periphery abs-sums of the two bands
                nc.vector.tensor_reduce(
                    pacc[:, 2 * i : 2 * i + 1],
                    d3[:, :, 0:CS],
                    axis=mybir.AxisListType.XY,
                    op=mybir.AluOpType.add,
                    apply_absolute_value=True,
                )
                nc.vector.tensor_reduce(
                    pacc[:, 2 * i + 1 : 2 * i + 2],
                    d3[:, :, CE:],
                    axis=mybir.AxisListType.XY,
                    op=mybir.AluOpType.add,
                    apply_absolute_value=True,
                )

            def body():
                diffs = {}
                base = 0
                for i, r in enumerate(sched):
                    emit_tile(i, r, base, diffs)
                    base += P * r
                for i in range(max(T - lag, 0), T):
                    emit_central(i, sched[i], diffs)

            if loop_n > 1:
                with tc.For_i(0, loop_n, 1):
                    body()
            else:
                body()

            # Tiny result DMAs on separate rings -> receipts in parallel.
            nc.sync.dma_start(cacc_out.ap(), cacc[:])
            nc.scalar.dma_start(pacc_out.ap(), pacc[:])

    nc.compile()
    return nc


_CACHED_NC = None


def _get_program() -> bass.Bass:
    global _CACHED_NC
    if _CACHED_NC is None:
        _CACHED_NC = build_program()
    return _CACHED_NC


def shard_inputs(pred: np.ndarray, target: np.ndarray) -> list[dict]:
    in_maps = []
    for i in range(N_CORES):
        sl = slice(i * B_SHARD, (i + 1) * B_SHARD)
        in_maps.append(
            {
                "pred": np.ascontiguousarray(pred[sl]).reshape(ROWS, W),
                "target": np.ascontiguousarray(target[sl]).reshape(ROWS, W),
            }
        )
    return in_maps


def reduce_partials(results: list[dict]) -> np.ndarray:
    tot_sq = 0.0
    tot_abs = 0.0
    for r in results:
        tot_sq += r["cacc"].astype(np.float64).sum()
        tot_abs += r["pacc"].astype(np.float64).sum()
    loss = tot_sq / (B * H * CW) + tot_abs / (B * H * PW)
    return np.asarray(loss, dtype=np.float32)


def kernel(pred: np.ndarray, target: np.ndarray) -> np.ndarray:
    pred = np.asarray(pred, dtype=np.float32)
    target = np.asarray(target, dtype=np.float32)
    nc = _get_program()
    in_maps = shard_inputs(pred, target)
    res = run_bass_kernel_spmd(nc, in_maps, list(range(N_CORES)))
    return reduce_partials(res.results)
